# revision 1
# baseline (speedup 1.0000x reference)
"""Trainium2 Bass kernel for nn_MultiHeadNetwork (moe_routing).

Strategy
--------
Host side (numpy, inside kernel()):
  * task id per row = argmax of the trailing one-hot block of x (data, not
    activation dependent), rows sorted by task id, batch split into 8
    contiguous 512-row chunks (one per NeuronCore).
  * Trunk weights replicated; per core the head only needs the few tasks its
    chunk spans.  The 512 columns are split into two 256-column windows; for
    each window the spanned tasks become "slots" (padded to the max SW over
    all cores/windows so the SPMD program structure is uniform; the slot
    *weights and masks* are per-core data).
  * Everything is pre-rounded to the fp32r grid (11 mantissa bits) and packed
    so every DMA is a contiguous [128, F] panel.

Device side (one SPMD Tile program on 8 cores):
  * Activations kept feature-major (hT: [feat partitions, batch free]) so each
    trunk layer is out = W_chunk.T @ hT with NO transposes anywhere.
  * All matmuls float32r (fp32 storage, full PE rate at N >= 256).
  * Trunk layers run k-OUTER over half-width (8 w-chunks -> 8 PSUM banks):
    the first matmul only needs one k-tile of activations + one weight panel,
    so the PE starts almost immediately and layer transitions don't stall.
  * ReLU + bias fused on the scalar engine straight out of PSUM.
  * Head: per (window, slot): psum = head_W[slot].T @ h3T[window cols], then
    copy_predicated with a host 0/1 mask selects the rows of that task.
  * Head bias and the inverse permutation are applied on host.
"""

import numpy as np
from contextlib import ExitStack

import concourse.bacc as bacc
import concourse.mybir as mybir
from concourse.tile import TileContext
from concourse import bass_utils

BATCH = 4096
FEAT = 2048
NUM_TASKS = 50
WIDTH = 2048
HEAD_DIM = 256
NCORES = 8
BPC = BATCH // NCORES          # 512 rows per core
NWIN = 2                       # head column windows per core
WINC = BPC // NWIN             # 256 columns per window
KIN = FEAT + NUM_TASKS         # 2098
KC0 = 17                       # input K chunks (zero-padded to 2176)
KPAD = KC0 * 128
KC = WIDTH // 128              # 16
WC = WIDTH // 128              # 16
HALF = WC // 2                 # 8 w-chunks per half-layer
QW = 4                         # w-chunks per trunk quarter (4 PSUM banks)
NQ = WC // QW                  # 4 quarters per layer
MH = HEAD_DIM // 128           # 2 head-dim halves

F32 = mybir.dt.float32
F32R = mybir.dt.float32r
U8 = mybir.dt.uint8

_PROG_CACHE: dict = {}


def round_fp32r(a: np.ndarray) -> np.ndarray:
    """Round fp32 to the fp32r grid (11 mantissa bits, RNE) like the HW does."""
    b = np.ascontiguousarray(a, np.float32).view(np.uint32)
    bias = np.uint32(0x7FF) + ((b >> np.uint32(12)) & np.uint32(1))
    out = (b + bias) & np.uint32(0xFFFFF000)
    return out.view(np.float32)


def _build(S: int, repeat: int = 1):
    """Build + compile the SPMD Tile program.  S = head slots per window.

    repeat > 1 wraps the whole body in a hardware For_i loop (benchmarking
    only: amortizes launch/RPC overhead across repeat executions).
    """
    nslot = NWIN * S
    nc = bacc.Bacc("TRN2", target_bir_lowering=False, debug=False)
    xT = nc.dram_tensor("xT", [KC0, 128, BPC], F32R, kind="ExternalInput").ap()
    w0 = nc.dram_tensor("w0p", [NQ, KC0, 128, QW * 128], F32R, kind="ExternalInput").ap()
    w1 = nc.dram_tensor("w1p", [NQ, KC, 128, QW * 128], F32R, kind="ExternalInput").ap()
    w2 = nc.dram_tensor("w2p", [NQ, KC, 128, QW * 128], F32R, kind="ExternalInput").ap()
    bia = nc.dram_tensor("bias", [128, 3 * WC], F32, kind="ExternalInput").ap()
    hws = nc.dram_tensor("hws", [nslot, 128, KC * HEAD_DIM], F32R, kind="ExternalInput").ap()
    msk = nc.dram_tensor("msk", [128, nslot * WINC], U8, kind="ExternalInput").ap()
    out = nc.dram_tensor("outT", [MH, 128, BPC], F32, kind="ExternalOutput").ap()

    with TileContext(nc) as tc, ExitStack() as ctx:
        # xT (17 tiles) and h2 (16) share slots: h2 allocates only after
        # layer 0 fully finished reading xT.  h3 gets its own pool (h1 is
        # still being read while h3 is produced).
        actA = ctx.enter_context(tc.tile_pool(name="actA", bufs=KC0))
        actB = ctx.enter_context(tc.tile_pool(name="actB", bufs=KC))
        wp = ctx.enter_context(tc.tile_pool(name="wp", bufs=12))
        cons = ctx.enter_context(tc.tile_pool(name="cons", bufs=1))
        hwp = ctx.enter_context(tc.tile_pool(name="hwp", bufs=5))
        op = ctx.enter_context(tc.tile_pool(name="op", bufs=MH))
        psp = ctx.enter_context(tc.tile_pool(name="psp", bufs=8, space="PSUM"))

        if repeat > 1:
            ctx.enter_context(tc.For_i(0, repeat, 1))

        bt = cons.tile([128, 3 * WC], F32, tag="bt")
        nc.sync.dma_start(bt[:], bia)
        mt = cons.tile([128, nslot * WINC], U8, tag="mt")
        nc.sync.dma_start(mt[:], msk)

        xt = [None] * KC0

        def trunk_layer(src, wdram, nk, li, pool, tag, load_x=False):
            outs = [None] * WC
            for q in range(NQ):
                pss = [
                    psp.tile([128, BPC], F32, tag="ps", name=f"psL{li}q{q}w{w}")
                    for w in range(QW)
                ]
                for k in range(nk):
                    wt = wp.tile([128, QW * 128], F32R, tag="wp", name=f"wtL{li}q{q}k{k}")
                    nc.sync.dma_start(wt[:], wdram[q, k])
                    if load_x and q == 0:
                        t = actA.tile([128, BPC], F32R, tag="actA", name=f"xt{k}")
                        nc.sync.dma_start(t[:], xT[k])
                        src[k] = t
                    for w in range(QW):
                        nc.tensor.matmul(
                            pss[w][:],
                            wt[:, w * 128:(w + 1) * 128],
                            src[k][:],
                            start=(k == 0),
                            stop=(k == nk - 1),
                        )
                for w in range(QW):
                    wc_i = q * QW + w
                    h = pool.tile([128, BPC], F32R, tag=tag, name=f"h{li}_{wc_i}")
                    nc.scalar.activation(
                        h[:], pss[w][:], mybir.ActivationFunctionType.Relu,
                        bias=bt[:, li * WC + wc_i: li * WC + wc_i + 1],
                    )
                    outs[wc_i] = h
            return outs

        h1 = trunk_layer(xt, w0, KC0, 0, actB, "actB", load_x=True)
        h2 = trunk_layer(h1, w1, KC, 1, actA, "actA")
        h3 = trunk_layer(h2, w2, KC, 2, actB, "actB")

        om = [op.tile([128, BPC], F32, tag="op", name=f"om{m}") for m in range(MH)]
        for win in range(NWIN):
            cols = slice(win * WINC, (win + 1) * WINC)
            for s in range(S):
                sl = win * S + s
                hw = hwp.tile([128, KC * HEAD_DIM], F32R, tag="hwp", name=f"hw{sl}")
                nc.sync.dma_start(hw[:], hws[sl])
                for m in range(MH):
                    ps = psp.tile([128, WINC], F32, tag="ps", name=f"psH{sl}m{m}")
                    for k in range(KC):
                        nc.tensor.matmul(
                            ps[:],
                            hw[:, k * HEAD_DIM + m * 128: k * HEAD_DIM + (m + 1) * 128],
                            h3[k][:, cols],
                            start=(k == 0),
                            stop=(k == KC - 1),
                        )
                    if s == 0:
                        nc.vector.tensor_copy(om[m][:, cols], ps[:])
                    else:
                        nc.vector.copy_predicated(
                            om[m][:, cols], mt[:, sl * WINC:(sl + 1) * WINC], ps[:]
                        )
            for m in range(MH):
                nc.sync.dma_start(out[m][:, cols], om[m][:, cols])

    nc.compile()
    return nc


def _pack_w(W, nk):
    # [NQ, nk, 128, QW*128]; [q, k, kp, w*128+m] = W[k*128+kp, (q*QW+w)*128+m]
    return np.ascontiguousarray(
        W.reshape(nk, 128, NQ, QW * 128).transpose(2, 0, 1, 3)
    )


def _pack_trunk(W0, W1, W2, b0, b1, b2):
    W0pad = np.zeros((KPAD, WIDTH), np.float32)
    W0pad[:KIN] = round_fp32r(W0)
    w0p = _pack_w(W0pad, KC0)
    w1p = _pack_w(round_fp32r(W1), KC)
    w2p = _pack_w(round_fp32r(W2), KC)
    bias = np.zeros((128, 3 * WC), np.float32)
    for li, b in enumerate((b0, b1, b2)):
        bias[:, li * WC:(li + 1) * WC] = b.reshape(WC, 128).T
    return w0p, w1p, w2p, bias


def prepare(x, W0, b0, W1, b1, W2, b2, head_W, head_b):
    """Host-side sharding. Returns (in_maps, order, sorted_task_ids, S)."""
    x = np.asarray(x, np.float32)
    W0 = np.asarray(W0, np.float32)
    W1 = np.asarray(W1, np.float32)
    W2 = np.asarray(W2, np.float32)
    b0 = np.asarray(b0, np.float32)
    b1 = np.asarray(b1, np.float32)
    b2 = np.asarray(b2, np.float32)
    head_W = np.asarray(head_W, np.float32)

    tid = np.argmax(x[:, -NUM_TASKS:], axis=1)
    order = np.argsort(tid, kind="stable")
    x_s = x[order]
    t_s = tid[order]

    # per (core, window) spanned task lists
    win_tasks = []   # [core][win] -> list of tasks
    for c in range(NCORES):
        per_win = []
        for w in range(NWIN):
            lo = c * BPC + w * WINC
            ch = t_s[lo: lo + WINC]
            per_win.append(list(dict.fromkeys(ch.tolist())))
        win_tasks.append(per_win)
    S = max(len(tl) for per in win_tasks for tl in per)

    w0p, w1p, w2p, bias = _pack_trunk(W0, W1, W2, b0, b1, b2)
    head_W = round_fp32r(head_W)
    # hw_pack[t, kp, kc*256 + j] = head_W[t, kc*128 + kp, j]
    hw_pack = np.ascontiguousarray(
        head_W.reshape(NUM_TASKS, KC, 128, HEAD_DIM)
        .transpose(0, 2, 1, 3)
        .reshape(NUM_TASKS, 128, KC * HEAD_DIM)
    )

    nslot = NWIN * S
    in_maps = []
    for c in range(NCORES):
        xs = x_s[c * BPC:(c + 1) * BPC]
        xTp = np.zeros((KPAD, BPC), np.float32)
        xTp[:KIN] = round_fp32r(xs.T)
        slot_tasks = []
        msk_c = np.zeros((128, nslot * WINC), np.uint8)
        for w in range(NWIN):
            tl = win_tasks[c][w]
            tl_p = tl + [tl[-1]] * (S - len(tl))
            lo = c * BPC + w * WINC
            ch = t_s[lo: lo + WINC]
            for s, t in enumerate(tl_p):
                sl = w * S + s
                slot_tasks.append(t)
                if 0 < s < len(tl):
                    msk_c[:, sl * WINC:(sl + 1) * WINC] = (ch == t)[None, :].astype(np.uint8)
        hws_c = np.ascontiguousarray(hw_pack[np.asarray(slot_tasks)])
        in_maps.append({
            "xT": np.ascontiguousarray(xTp.reshape(KC0, 128, BPC)),
            "w0p": w0p, "w1p": w1p, "w2p": w2p, "bias": bias,
            "hws": hws_c, "msk": msk_c,
        })
    return in_maps, order, t_s, S


def _assemble(results, order, t_s, head_b):
    head_b = np.asarray(head_b, np.float32)
    outs = []
    for c in range(NCORES):
        oT = results[c]["outT"]                       # [MH, 128, BPC]
        outs.append(oT.reshape(HEAD_DIM, BPC).T)      # [BPC, 256]
    out_s = np.concatenate(outs, axis=0) + head_b[t_s]
    out = np.empty_like(out_s)
    out[order] = out_s
    return out.astype(np.float32)


def kernel(x, W0, b0, W1, b1, W2, b2, head_W, head_b):
    in_maps, order, t_s, S = prepare(x, W0, b0, W1, b1, W2, b2, head_W, head_b)
    nc = _PROG_CACHE.get(S)
    if nc is None:
        nc = _build(S)
        _PROG_CACHE[S] = nc
    res = bass_utils.run_bass_kernel_spmd(nc, in_maps, core_ids=list(range(NCORES)))
    return _assemble(res.results, order, t_s, head_b)



# revision 2
# speedup vs baseline: 1.0938x; 1.0938x over previous
"""Trainium2 Bass kernel for nn_MultiHeadNetwork (moe_routing) — v4.

On top of kernel3 (bf16, batched weight DMA, resident head weights):
  * Packing heuristics that reliably reach S=4 head slots per 256-col
    window (sorted order gives S=5): several greedy sequencers are tried,
    best S wins.  Head matmuls drop 320 -> 256 per core.
  * One-hot fold: the trailing 50 one-hot features of x contribute
    W0[2048+tid] per row — a per-column bias.  Host gathers that map and the
    device adds it to the layer-0 PSUM on the vector engine, dropping the
    17th k-chunk of layer 0 (16 matmuls/core).
"""

import numpy as np
import ml_dtypes
from contextlib import ExitStack

import concourse.bacc as bacc
import concourse.mybir as mybir
from concourse.tile import TileContext
from concourse import bass_utils

BATCH = 4096
FEAT = 2048
NUM_TASKS = 50
WIDTH = 2048
HEAD_DIM = 256
NCORES = 8
BPC = BATCH // NCORES          # 512 rows per core
NWIN = 2                       # head column windows per core
WINC = BPC // NWIN             # 256 columns per window
NWTOT = NCORES * NWIN          # 16 windows
KC = WIDTH // 128              # 16 (also layer-0 k-chunks after fold)
WC = WIDTH // 128              # 16
QW = 4                         # w-chunks per trunk quarter (4 PSUM banks)
NQ = WC // QW                  # 4 quarters per layer
MH = HEAD_DIM // 128           # 2 head-dim halves

F32 = mybir.dt.float32
BF16 = mybir.dt.bfloat16
U8 = mybir.dt.uint8
NPBF16 = ml_dtypes.bfloat16

_PROG_CACHE: dict = {}


def _build(S: int, repeat: int = 1):
    """Build + compile the SPMD Tile program.  S = head slots per window."""
    nslot = NWIN * S
    nc = bacc.Bacc("TRN2", target_bir_lowering=False, debug=False)
    xT = nc.dram_tensor("xT", [KC, 128, BPC], BF16, kind="ExternalInput").ap()
    w0 = nc.dram_tensor("w0p", [NQ, 128, KC * 512], BF16, kind="ExternalInput").ap()
    w1 = nc.dram_tensor("w1p", [NQ, 128, KC * 512], BF16, kind="ExternalInput").ap()
    w2 = nc.dram_tensor("w2p", [NQ, 128, KC * 512], BF16, kind="ExternalInput").ap()
    ohm = nc.dram_tensor("ohm", [WC, 128, BPC], BF16, kind="ExternalInput").ap()
    bia = nc.dram_tensor("bias", [128, 3 * WC], F32, kind="ExternalInput").ap()
    hws = nc.dram_tensor("hws", [nslot, 128, KC * HEAD_DIM], BF16, kind="ExternalInput").ap()
    msk = nc.dram_tensor("msk", [128, nslot * WINC], U8, kind="ExternalInput").ap()
    out = nc.dram_tensor("outT", [MH, 128, BPC], F32, kind="ExternalOutput").ap()

    with TileContext(nc) as tc, ExitStack() as ctx:
        actA = ctx.enter_context(tc.tile_pool(name="actA", bufs=KC))
        actB = ctx.enter_context(tc.tile_pool(name="actB", bufs=KC))
        wp = ctx.enter_context(tc.tile_pool(name="wp", bufs=4))
        ohp = ctx.enter_context(tc.tile_pool(name="ohp", bufs=WC))
        cons = ctx.enter_context(tc.tile_pool(name="cons", bufs=1))
        hwp = ctx.enter_context(tc.tile_pool(name="hwp", bufs=nslot))
        op = ctx.enter_context(tc.tile_pool(name="op", bufs=MH))
        psp = ctx.enter_context(tc.tile_pool(name="psp", bufs=8, space="PSUM"))

        if repeat > 1:
            ctx.enter_context(tc.For_i(0, repeat, 1))

        bt = cons.tile([128, 3 * WC], F32, tag="bt")
        nc.sync.dma_start(bt[:], bia)
        mt = cons.tile([128, nslot * WINC], U8, tag="mt")
        nc.sync.dma_start(mt[:], msk)

        xt = [None] * KC
        oht = [None] * WC

        def trunk_layer(src, wdram, li, pool, tag, load_x=False):
            outs = [None] * WC
            nk = KC
            nkA = nk // 2
            for q in range(NQ):
                wtA = wp.tile([128, nkA * 512], BF16, tag="wp", name=f"wA{li}q{q}")
                nc.sync.dma_start(wtA[:], wdram[q, :, : nkA * 512])
                wtB = wp.tile([128, (nk - nkA) * 512], BF16, tag="wp", name=f"wB{li}q{q}")
                nc.sync.dma_start(wtB[:], wdram[q, :, nkA * 512:])
                pss = [
                    psp.tile([128, BPC], F32, tag="ps", name=f"psL{li}q{q}w{w}")
                    for w in range(QW)
                ]
                for k in range(nk):
                    if load_x and q == 0:
                        t = actA.tile([128, BPC], BF16, tag="actA", name=f"xt{k}")
                        nc.sync.dma_start(t[:], xT[k])
                        src[k] = t
                        o = ohp.tile([128, BPC], BF16, tag="ohp", name=f"oh{k}")
                        nc.sync.dma_start(o[:], ohm[k])
                        oht[k] = o
                    wt, kk = (wtA, k) if k < nkA else (wtB, k - nkA)
                    for w in range(QW):
                        nc.tensor.matmul(
                            pss[w][:],
                            wt[:, kk * 512 + w * 128: kk * 512 + (w + 1) * 128],
                            src[k][:],
                            start=(k == 0),
                            stop=(k == nk - 1),
                        )
                for w in range(QW):
                    wc_i = q * QW + w
                    if li == 0:
                        nc.vector.tensor_add(pss[w][:], pss[w][:], oht[wc_i][:])
                    h = pool.tile([128, BPC], BF16, tag=tag, name=f"h{li}_{wc_i}")
                    nc.scalar.activation(
                        h[:], pss[w][:], mybir.ActivationFunctionType.Relu,
                        bias=bt[:, li * WC + wc_i: li * WC + wc_i + 1],
                    )
                    outs[wc_i] = h
            return outs

        h1 = trunk_layer(xt, w0, 0, actB, "actB", load_x=True)
        h2 = trunk_layer(h1, w1, 1, actA, "actA")

        # issue head-weight DMAs before layer 2 so they overlap its compute
        hwt = []
        for sl in range(nslot):
            hw = hwp.tile([128, KC * HEAD_DIM], BF16, tag="hwp", name=f"hw{sl}")
            nc.sync.dma_start(hw[:], hws[sl])
            hwt.append(hw)

        h3 = trunk_layer(h2, w2, 2, actB, "actB")

        om = [op.tile([128, BPC], F32, tag="op", name=f"om{m}") for m in range(MH)]
        for win in range(NWIN):
            cols = slice(win * WINC, (win + 1) * WINC)
            for s in range(S):
                sl = win * S + s
                hw = hwt[sl]
                for m in range(MH):
                    ps = psp.tile([128, WINC], F32, tag="ps", name=f"psH{sl}m{m}")
                    for k in range(KC):
                        nc.tensor.matmul(
                            ps[:],
                            hw[:, k * HEAD_DIM + m * 128: k * HEAD_DIM + (m + 1) * 128],
                            h3[k][:, cols],
                            start=(k == 0),
                            stop=(k == KC - 1),
                        )
                    if s == 0:
                        nc.vector.tensor_copy(om[m][:, cols], ps[:])
                    else:
                        nc.vector.copy_predicated(
                            om[m][:, cols], mt[:, sl * WINC:(sl + 1) * WINC], ps[:]
                        )
            for m in range(MH):
                nc.sync.dma_start(out[m][:, cols], om[m][:, cols])

    nc.compile()
    return nc


def _pack_w(W):
    # [NQ, 128, KC*512]; [q, p, k*512 + j] = W[k*128 + p, q*512 + j]
    return np.ascontiguousarray(
        W.reshape(KC, 128, NQ, 512).transpose(2, 1, 0, 3).reshape(NQ, 128, KC * 512)
    )


def _seq_pack(sizes, variant):
    """Sequence (task, nrows) chunks so each 256-row window spans few tasks.

    Returns list of windows, each a list of (task, nrows)."""
    remaining = {t: s for t, s in enumerate(sizes) if s > 0}
    windows = []
    carry = None  # (task, nrows left)
    for w in range(NWTOT):
        cap = WINC
        cur = []
        if carry is not None:
            t, n = carry
            take = min(cap, n)
            cur.append((t, take))
            cap -= take
            carry = (t, n - take) if n - take else None
        # place whole tasks
        while cap > 0 and carry is None:
            avail = [t for t, s in remaining.items() if s <= cap]
            exact = [t for t in avail if remaining[t] == cap]
            if exact:
                t = exact[0]
            elif avail and len(cur) < 3:
                if variant == "big":
                    t = max(avail, key=lambda t: remaining[t])
                else:
                    t = min(avail, key=lambda t: remaining[t])
            else:
                t = None
            if t is not None:
                cur.append((t, remaining.pop(t)))
                cap -= cur[-1][1]
            else:
                break
        if cap > 0:
            # split-fill from the largest remaining task
            t = max(remaining, key=lambda t: remaining[t])
            n = remaining.pop(t)
            take = min(cap, n)
            cur.append((t, take))
            cap -= take
            if n - take:
                carry = (t, n - take)
        assert cap == 0, (w, cap)
        windows.append(cur)
    assert carry is None and not remaining
    return windows


def _balance(tid):
    """Minimize S = max distinct tasks per 256-row window.

    Returns (order, win_tasks) like kernel3."""
    sizes = np.bincount(tid, minlength=NUM_TASKS)
    best = None
    for variant in ("small", "big"):
        try:
            ws = _seq_pack(sizes, variant)
        except AssertionError:
            continue
        s = max(len(w) for w in ws)
        if best is None or s < best[0]:
            best = (s, ws)
    # fallback: plain sorted order
    t_sorted = np.sort(tid)
    ws_sorted = []
    for w in range(NWTOT):
        ch = t_sorted[w * WINC:(w + 1) * WINC]
        tl, cnts = np.unique(ch, return_counts=True)
        # preserve appearance order
        seen = list(dict.fromkeys(ch.tolist()))
        ws_sorted.append([(t, int((ch == t).sum())) for t in seen])
    s_sorted = max(len(w) for w in ws_sorted)
    if best is None or s_sorted < best[0]:
        best = (s_sorted, ws_sorted)

    _, windows = best
    rows_by_task = {t: list(np.nonzero(tid == t)[0]) for t in range(NUM_TASKS)}
    ptr = {t: 0 for t in range(NUM_TASKS)}
    order = []
    win_tasks = []
    for w in range(NWTOT):
        tl = []
        for t, n in windows[w]:
            p = ptr[t]
            order.extend(rows_by_task[t][p:p + n])
            ptr[t] = p + n
            tl.append(t)
        win_tasks.append(tl)
    return np.asarray(order), win_tasks


def prepare(x, W0, b0, W1, b1, W2, b2, head_W, head_b):
    """Host-side sharding. Returns (in_maps, order, sorted_task_ids, S)."""
    x = np.asarray(x, np.float32)
    W0 = np.asarray(W0, np.float32)
    W1 = np.asarray(W1, np.float32)
    W2 = np.asarray(W2, np.float32)
    b0 = np.asarray(b0, np.float32)
    b1 = np.asarray(b1, np.float32)
    b2 = np.asarray(b2, np.float32)
    head_W = np.asarray(head_W, np.float32)

    tid = np.argmax(x[:, -NUM_TASKS:], axis=1)
    order, win_tasks_flat = _balance(tid)
    x_s = x[order]
    t_s = tid[order]

    win_tasks = [
        [win_tasks_flat[c * NWIN + w] for w in range(NWIN)] for c in range(NCORES)
    ]
    S = max(len(tl) for per in win_tasks for tl in per)

    w0p = _pack_w(W0[:FEAT].astype(NPBF16))
    w1p = _pack_w(W1.astype(NPBF16))
    w2p = _pack_w(W2.astype(NPBF16))
    bias = np.zeros((128, 3 * WC), np.float32)
    for li, b in enumerate((b0, b1, b2)):
        bias[:, li * WC:(li + 1) * WC] = b.reshape(WC, 128).T

    # one-hot fold: per-row bias row W0[FEAT + tid], laid out like the
    # layer-0 psum [w-chunk, 128, col]
    W0oh16 = W0[FEAT:].astype(NPBF16)          # [NUM_TASKS, WIDTH]

    head_W16 = head_W.astype(NPBF16)
    hw_pack = np.ascontiguousarray(
        head_W16.reshape(NUM_TASKS, KC, 128, HEAD_DIM)
        .transpose(0, 2, 1, 3)
        .reshape(NUM_TASKS, 128, KC * HEAD_DIM)
    )

    nslot = NWIN * S
    in_maps = []
    for c in range(NCORES):
        rows = slice(c * BPC, (c + 1) * BPC)
        xs = x_s[rows]
        xTp = np.ascontiguousarray(
            xs[:, :FEAT].T.astype(NPBF16).reshape(KC, 128, BPC)
        )
        ohm_c = np.ascontiguousarray(
            W0oh16[t_s[rows]].T.reshape(WC, 128, BPC)
        )
        slot_tasks = []
        msk_c = np.zeros((128, nslot * WINC), np.uint8)
        for w in range(NWIN):
            tl = win_tasks[c][w]
            tl_p = tl + [tl[-1]] * (S - len(tl))
            lo = c * BPC + w * WINC
            ch = t_s[lo: lo + WINC]
            for s, t in enumerate(tl_p):
                sl = w * S + s
                slot_tasks.append(t)
                if 0 < s < len(tl):
                    msk_c[:, sl * WINC:(sl + 1) * WINC] = (ch == t)[None, :].astype(np.uint8)
        hws_c = np.ascontiguousarray(hw_pack[np.asarray(slot_tasks)])
        in_maps.append({
            "xT": xTp, "ohm": ohm_c,
            "w0p": w0p, "w1p": w1p, "w2p": w2p, "bias": bias,
            "hws": hws_c, "msk": msk_c,
        })
    return in_maps, order, t_s, S


def _assemble(results, order, t_s, head_b):
    head_b = np.asarray(head_b, np.float32)
    outs = []
    for c in range(NCORES):
        oT = results[c]["outT"]                       # [MH, 128, BPC]
        outs.append(oT.reshape(HEAD_DIM, BPC).T)      # [BPC, 256]
    out_s = np.concatenate(outs, axis=0) + head_b[t_s]
    out = np.empty_like(out_s)
    out[order] = out_s
    return out.astype(np.float32)


def kernel(x, W0, b0, W1, b1, W2, b2, head_W, head_b):
    in_maps, order, t_s, S = prepare(x, W0, b0, W1, b1, W2, b2, head_W, head_b)
    nc = _PROG_CACHE.get(S)
    if nc is None:
        nc = _build(S)
        _PROG_CACHE[S] = nc
    res = bass_utils.run_bass_kernel_spmd(nc, in_maps, core_ids=list(range(NCORES)))
    return _assemble(res.results, order, t_s, head_b)


# revision 3
# speedup vs baseline: 1.1121x; 1.0167x over previous
"""Trainium2 Bass kernel for nn_MultiHeadNetwork (moe_routing) — v4.

On top of kernel3 (bf16, batched weight DMA, resident head weights):
  * Packing heuristics that reliably reach S=4 head slots per 256-col
    window (sorted order gives S=5): several greedy sequencers are tried,
    best S wins.  Head matmuls drop 320 -> 256 per core.
  * One-hot fold: the trailing 50 one-hot features of x contribute
    W0[2048+tid] per row — a per-column bias.  Host gathers that map and the
    device adds it to the layer-0 PSUM on the vector engine, dropping the
    17th k-chunk of layer 0 (16 matmuls/core).

v5: tiered head-slot widths.  Within each 256-col window the tasks are
ordered by descending row count; slot s's task region then provably lies in
cols [64*s, 256), so slot matmuls can use N = 256-64*s (256/192/128/64)
instead of 4x256 — a 37.5% cut in head streamed columns (PE time here is
~21ns + N*0.527ns per matmul; the clock sits at ~1.9GHz under sustained
load, weight loads are already hidden).

v6: data-derived slot bounds.  Every slot (including rank 0) is a masked
copy, so slot rank s only needs to cover [min_w start, max_w end) over the
real windows — measured ~450 cols total per window instead of 640.  The
program is cached per (S, bounds).
"""

import numpy as np
import ml_dtypes
from contextlib import ExitStack

import concourse.bacc as bacc
import concourse.mybir as mybir
from concourse.tile import TileContext
from concourse import bass_utils

BATCH = 4096
FEAT = 2048
NUM_TASKS = 50
WIDTH = 2048
HEAD_DIM = 256
NCORES = 8
BPC = BATCH // NCORES          # 512 rows per core
NWIN = 2                       # head column windows per core
WINC = BPC // NWIN             # 256 columns per window
NWTOT = NCORES * NWIN          # 16 windows
KC = WIDTH // 128              # 16 (also layer-0 k-chunks after fold)
WC = WIDTH // 128              # 16
QW = 4                         # w-chunks per trunk quarter (4 PSUM banks)
NQ = WC // QW                  # 4 quarters per layer
MH = HEAD_DIM // 128           # 2 head-dim halves

F32 = mybir.dt.float32
BF16 = mybir.dt.bfloat16
U8 = mybir.dt.uint8
NPBF16 = ml_dtypes.bfloat16

_PROG_CACHE: dict = {}


_LAST_BOUNDS = None


def _build(S: int, repeat: int = 1, bounds=None):
    """Build + compile the SPMD Tile program.  S = head slots per window.

    bounds: per-slot-rank (lo, hi) column range within the window; None
    falls back to the last bounds prepare() derived, else full windows."""
    global _LAST_BOUNDS
    if bounds is None:
        bounds = _LAST_BOUNDS or tuple((0, WINC) for _ in range(S))
    nslot = NWIN * S
    nc = bacc.Bacc("TRN2", target_bir_lowering=False, debug=False)
    xT = nc.dram_tensor("xT", [KC, 128, BPC], BF16, kind="ExternalInput").ap()
    w0 = nc.dram_tensor("w0p", [NQ, 128, KC * 512], BF16, kind="ExternalInput").ap()
    w1 = nc.dram_tensor("w1p", [NQ, 128, KC * 512], BF16, kind="ExternalInput").ap()
    w2 = nc.dram_tensor("w2p", [NQ, 128, KC * 512], BF16, kind="ExternalInput").ap()
    ohm = nc.dram_tensor("ohm", [WC, 128, BPC], BF16, kind="ExternalInput").ap()
    bia = nc.dram_tensor("bias", [128, 3 * WC], F32, kind="ExternalInput").ap()
    hws = nc.dram_tensor("hws", [nslot, 128, KC * HEAD_DIM], BF16, kind="ExternalInput").ap()
    msk = nc.dram_tensor("msk", [128, nslot * WINC], U8, kind="ExternalInput").ap()
    out = nc.dram_tensor("outT", [MH, 128, BPC], F32, kind="ExternalOutput").ap()

    with TileContext(nc) as tc, ExitStack() as ctx:
        actA = ctx.enter_context(tc.tile_pool(name="actA", bufs=KC))
        actB = ctx.enter_context(tc.tile_pool(name="actB", bufs=KC))
        wp = ctx.enter_context(tc.tile_pool(name="wp", bufs=4))
        ohp = ctx.enter_context(tc.tile_pool(name="ohp", bufs=WC))
        cons = ctx.enter_context(tc.tile_pool(name="cons", bufs=1))
        hwp = ctx.enter_context(tc.tile_pool(name="hwp", bufs=nslot))
        op = ctx.enter_context(tc.tile_pool(name="op", bufs=MH))
        psp = ctx.enter_context(tc.tile_pool(name="psp", bufs=8, space="PSUM"))

        if repeat > 1:
            ctx.enter_context(tc.For_i(0, repeat, 1))

        bt = cons.tile([128, 3 * WC], F32, tag="bt")
        nc.sync.dma_start(bt[:], bia)
        mt = cons.tile([128, nslot * WINC], U8, tag="mt")
        nc.sync.dma_start(mt[:], msk)

        xt = [None] * KC
        oht = [None] * WC

        def trunk_layer(src, wdram, li, pool, tag, load_x=False):
            outs = [None] * WC
            nk = KC
            nkA = nk // 2
            for q in range(NQ):
                wtA = wp.tile([128, nkA * 512], BF16, tag="wp", name=f"wA{li}q{q}")
                nc.sync.dma_start(wtA[:], wdram[q, :, : nkA * 512])
                wtB = wp.tile([128, (nk - nkA) * 512], BF16, tag="wp", name=f"wB{li}q{q}")
                nc.sync.dma_start(wtB[:], wdram[q, :, nkA * 512:])
                pss = [
                    psp.tile([128, BPC], F32, tag="ps", name=f"psL{li}q{q}w{w}")
                    for w in range(QW)
                ]
                for k in range(nk):
                    if load_x and q == 0:
                        t = actA.tile([128, BPC], BF16, tag="actA", name=f"xt{k}")
                        nc.sync.dma_start(t[:], xT[k])
                        src[k] = t
                        o = ohp.tile([128, BPC], BF16, tag="ohp", name=f"oh{k}")
                        nc.sync.dma_start(o[:], ohm[k])
                        oht[k] = o
                    wt, kk = (wtA, k) if k < nkA else (wtB, k - nkA)
                    for w in range(QW):
                        nc.tensor.matmul(
                            pss[w][:],
                            wt[:, kk * 512 + w * 128: kk * 512 + (w + 1) * 128],
                            src[k][:],
                            start=(k == 0),
                            stop=(k == nk - 1),
                        )
                for w in range(QW):
                    wc_i = q * QW + w
                    if li == 0:
                        nc.vector.tensor_add(pss[w][:], pss[w][:], oht[wc_i][:])
                    h = pool.tile([128, BPC], BF16, tag=tag, name=f"h{li}_{wc_i}")
                    nc.scalar.activation(
                        h[:], pss[w][:], mybir.ActivationFunctionType.Relu,
                        bias=bt[:, li * WC + wc_i: li * WC + wc_i + 1],
                    )
                    outs[wc_i] = h
            return outs

        h1 = trunk_layer(xt, w0, 0, actB, "actB", load_x=True)
        h2 = trunk_layer(h1, w1, 1, actA, "actA")

        # issue head-weight DMAs before layer 2 so they overlap its compute
        hwt = []
        for sl in range(nslot):
            hw = hwp.tile([128, KC * HEAD_DIM], BF16, tag="hwp", name=f"hw{sl}")
            nc.sync.dma_start(hw[:], hws[sl])
            hwt.append(hw)

        h3 = trunk_layer(h2, w2, 2, actB, "actB")

        # every slot is a masked copy over its data-derived column range;
        # each window column belongs to exactly one task, so coverage is exact
        om = [op.tile([128, BPC], F32, tag="op", name=f"om{m}") for m in range(MH)]
        for win in range(NWIN):
            cols = slice(win * WINC, (win + 1) * WINC)
            for s in range(S):
                sl = win * S + s
                hw = hwt[sl]
                lo, hi = bounds[s]
                sw = hi - lo
                scols = slice(win * WINC + lo, win * WINC + hi)
                for m in range(MH):
                    ps = psp.tile([128, sw], F32, tag="ps", name=f"psH{sl}m{m}")
                    for k in range(KC):
                        nc.tensor.matmul(
                            ps[:],
                            hw[:, k * HEAD_DIM + m * 128: k * HEAD_DIM + (m + 1) * 128],
                            h3[k][:, scols],
                            start=(k == 0),
                            stop=(k == KC - 1),
                        )
                    nc.vector.copy_predicated(
                        om[m][:, scols],
                        mt[:, sl * WINC + lo: sl * WINC + hi],
                        ps[:],
                    )
            for m in range(MH):
                nc.sync.dma_start(out[m][:, cols], om[m][:, cols])

    nc.compile()
    return nc


def _pack_w(W):
    # [NQ, 128, KC*512]; [q, p, k*512 + j] = W[k*128 + p, q*512 + j]
    return np.ascontiguousarray(
        W.reshape(KC, 128, NQ, 512).transpose(2, 1, 0, 3).reshape(NQ, 128, KC * 512)
    )


def _seq_pack(sizes, variant):
    """Sequence (task, nrows) chunks so each 256-row window spans few tasks.

    Returns list of windows, each a list of (task, nrows)."""
    remaining = {t: s for t, s in enumerate(sizes) if s > 0}
    windows = []
    carry = None  # (task, nrows left)
    for w in range(NWTOT):
        cap = WINC
        cur = []
        if carry is not None:
            t, n = carry
            take = min(cap, n)
            cur.append((t, take))
            cap -= take
            carry = (t, n - take) if n - take else None
        # place whole tasks
        while cap > 0 and carry is None:
            avail = [t for t, s in remaining.items() if s <= cap]
            exact = [t for t in avail if remaining[t] == cap]
            if exact:
                t = exact[0]
            elif avail and len(cur) < 3:
                if variant == "big":
                    t = max(avail, key=lambda t: remaining[t])
                else:
                    t = min(avail, key=lambda t: remaining[t])
            else:
                t = None
            if t is not None:
                cur.append((t, remaining.pop(t)))
                cap -= cur[-1][1]
            else:
                break
        if cap > 0:
            # split-fill from the largest remaining task
            t = max(remaining, key=lambda t: remaining[t])
            n = remaining.pop(t)
            take = min(cap, n)
            cur.append((t, take))
            cap -= take
            if n - take:
                carry = (t, n - take)
        assert cap == 0, (w, cap)
        windows.append(cur)
    assert carry is None and not remaining
    return windows


def _balance(tid):
    """Minimize S = max distinct tasks per 256-row window.

    Returns (order, win_tasks) like kernel3."""
    sizes = np.bincount(tid, minlength=NUM_TASKS)
    best = None
    for variant in ("small", "big"):
        try:
            ws = _seq_pack(sizes, variant)
        except AssertionError:
            continue
        s = max(len(w) for w in ws)
        if best is None or s < best[0]:
            best = (s, ws)
    # fallback: plain sorted order
    t_sorted = np.sort(tid)
    ws_sorted = []
    for w in range(NWTOT):
        ch = t_sorted[w * WINC:(w + 1) * WINC]
        tl, cnts = np.unique(ch, return_counts=True)
        # preserve appearance order
        seen = list(dict.fromkeys(ch.tolist()))
        ws_sorted.append([(t, int((ch == t).sum())) for t in seen])
    s_sorted = max(len(w) for w in ws_sorted)
    if best is None or s_sorted < best[0]:
        best = (s_sorted, ws_sorted)

    _, windows = best
    rows_by_task = {t: list(np.nonzero(tid == t)[0]) for t in range(NUM_TASKS)}
    ptr = {t: 0 for t in range(NUM_TASKS)}
    order = []
    win_tasks = []
    for w in range(NWTOT):
        tl = []
        # desc by in-window size: slot s's region then starts at col >= 64*s
        for t, n in sorted(windows[w], key=lambda tn: -tn[1]):
            p = ptr[t]
            order.extend(rows_by_task[t][p:p + n])
            ptr[t] = p + n
            tl.append(t)
        win_tasks.append(tl)
    return np.asarray(order), win_tasks


def prepare(x, W0, b0, W1, b1, W2, b2, head_W, head_b):
    """Host-side sharding. Returns (in_maps, order, sorted_task_ids, S)."""
    x = np.asarray(x, np.float32)
    W0 = np.asarray(W0, np.float32)
    W1 = np.asarray(W1, np.float32)
    W2 = np.asarray(W2, np.float32)
    b0 = np.asarray(b0, np.float32)
    b1 = np.asarray(b1, np.float32)
    b2 = np.asarray(b2, np.float32)
    head_W = np.asarray(head_W, np.float32)

    tid = np.argmax(x[:, -NUM_TASKS:], axis=1)
    order, win_tasks_flat = _balance(tid)
    x_s = x[order]
    t_s = tid[order]

    win_tasks = [
        [win_tasks_flat[c * NWIN + w] for w in range(NWIN)] for c in range(NCORES)
    ]
    S = max(len(tl) for per in win_tasks for tl in per)

    w0p = _pack_w(W0[:FEAT].astype(NPBF16))
    w1p = _pack_w(W1.astype(NPBF16))
    w2p = _pack_w(W2.astype(NPBF16))
    bias = np.zeros((128, 3 * WC), np.float32)
    for li, b in enumerate((b0, b1, b2)):
        bias[:, li * WC:(li + 1) * WC] = b.reshape(WC, 128).T

    # one-hot fold: per-row bias row W0[FEAT + tid], laid out like the
    # layer-0 psum [w-chunk, 128, col]
    W0oh16 = W0[FEAT:].astype(NPBF16)          # [NUM_TASKS, WIDTH]

    head_W16 = head_W.astype(NPBF16)
    hw_pack = np.ascontiguousarray(
        head_W16.reshape(NUM_TASKS, KC, 128, HEAD_DIM)
        .transpose(0, 2, 1, 3)
        .reshape(NUM_TASKS, 128, KC * HEAD_DIM)
    )

    # data-derived per-slot-rank column bounds over all real windows
    lo_s = [WINC] * S
    hi_s = [0] * S
    for w in range(NWTOT):
        ch = t_s[w * WINC:(w + 1) * WINC]
        for s, t in enumerate(win_tasks_flat[w]):
            pos = np.nonzero(ch == t)[0]
            lo_s[s] = min(lo_s[s], int(pos.min()))
            hi_s[s] = max(hi_s[s], int(pos.max()) + 1)
    bounds = tuple(
        (lo, hi) if lo < hi else (0, WINC) for lo, hi in zip(lo_s, hi_s)
    )
    global _LAST_BOUNDS
    _LAST_BOUNDS = bounds

    nslot = NWIN * S
    in_maps = []
    for c in range(NCORES):
        rows = slice(c * BPC, (c + 1) * BPC)
        xs = x_s[rows]
        xTp = np.ascontiguousarray(
            xs[:, :FEAT].T.astype(NPBF16).reshape(KC, 128, BPC)
        )
        ohm_c = np.ascontiguousarray(
            W0oh16[t_s[rows]].T.reshape(WC, 128, BPC)
        )
        slot_tasks = []
        msk_c = np.zeros((128, nslot * WINC), np.uint8)
        for w in range(NWIN):
            tl = win_tasks[c][w]
            tl_p = tl + [tl[-1]] * (S - len(tl))
            lo = c * BPC + w * WINC
            ch = t_s[lo: lo + WINC]
            for s, t in enumerate(tl_p):
                sl = w * S + s
                slot_tasks.append(t)
                if s < len(tl):
                    msk_c[:, sl * WINC:(sl + 1) * WINC] = (ch == t)[None, :].astype(np.uint8)
        hws_c = np.ascontiguousarray(hw_pack[np.asarray(slot_tasks)])
        in_maps.append({
            "xT": xTp, "ohm": ohm_c,
            "w0p": w0p, "w1p": w1p, "w2p": w2p, "bias": bias,
            "hws": hws_c, "msk": msk_c,
        })
    return in_maps, order, t_s, S


def _assemble(results, order, t_s, head_b):
    head_b = np.asarray(head_b, np.float32)
    outs = []
    for c in range(NCORES):
        oT = results[c]["outT"]                       # [MH, 128, BPC]
        outs.append(oT.reshape(HEAD_DIM, BPC).T)      # [BPC, 256]
    out_s = np.concatenate(outs, axis=0) + head_b[t_s]
    out = np.empty_like(out_s)
    out[order] = out_s
    return out.astype(np.float32)


def kernel(x, W0, b0, W1, b1, W2, b2, head_W, head_b):
    in_maps, order, t_s, S = prepare(x, W0, b0, W1, b1, W2, b2, head_W, head_b)
    key = (S, _LAST_BOUNDS)
    nc = _PROG_CACHE.get(key)
    if nc is None:
        nc = _build(S, bounds=_LAST_BOUNDS)
        _PROG_CACHE[key] = nc
    res = bass_utils.run_bass_kernel_spmd(nc, in_maps, core_ids=list(range(NCORES)))
    return _assemble(res.results, order, t_s, head_b)
